# revision 15
# baseline (speedup 1.0000x reference)
"""Trainium2 Bass kernel for CartNN minimal-NEAT forward pass.

Computes out = tanh(tanh(x @ w + b))[:, None] for x [16384, 4096] f32,
w [4096] f32, b [1] f32, data-parallel across 8 NeuronCores (2048 batch
rows per core). Memory-bound: each core streams its 32 MiB x shard once.

Per-core structure (measured on HW, iterated via NTFF profiles):
  - x streams as 16 [128, 4096] tiles split alternately across the two
    physical HWDGE rings (even tiles sync/qSPDynamicHW, odd tiles
    scalar/qActDynamicHW). All x DMAs write full 128-partition tiles:
    partition-sliced destinations (e.g. [0:120]) measurably halve the
    per-descriptor SDMA rate (port-swizzle misalignment), which is why
    engine-15 starvation via 120-row tiles regressed 117 -> 175 us.
  - SDMA engine 15 is ~20% slower than engines 0-14 for sync-ring
    traffic (known HW quirk), and with uniform tiles it carries 1/16
    of the bytes, so an all-sync stream is engine-15-bound: ~98 us
    busy vs ~79.5 for the rest. The ring split tests whether the
    slowdown is per-ring (descriptor-fetch port contention).
  - w is loaded once (16 KiB, scalar ring) and broadcast to all 128
    partitions by TensorE outer products ones[128,1] @ w[1,512]
    (PSUM->SBUF copies on ScalarE): zero extra HBM traffic and no
    sync-ring involvement (stride-0 DRAM broadcast DMAs measurably
    poison the x stream).
  - The dot product is one fused mul+reduce VectorE op per tile
    (affine_mul_reduce, ~4.5 us; TensorTensorReduce crashes the
    device). The first 4 tiles are split along K with staggered
    emission so DVE starts before the w broadcast completes; their
    quarter-partials are folded into acc right after the stagger (DVE
    program order) so the mid-stream output chunk doesn't wait.
  - Tiles 8 and 10 are offloaded off VectorE: GpSimd multiplies,
    ScalarE reduces via activation-accum. With the ~6 us/tile
    engine-15-paced arrival rate DVE then idles between tiles instead
    of being backlogged when the last x bytes land.
  - Output is emitted in two chunks: tiles 0..13 go tanh(tanh(.+b)) ->
    TensorE transpose -> [14,128] DMA mid-stream (fully hidden under
    the x stream tail); the end-of-kernel chain is only tiles 14/15:
    final quarter affines, 3 adds, tanh x2 on [128,2], transpose, one
    1 KiB DMA of 512-B rows from the scalar ring (ScalarE just wrote
    the data, skipping the ScalarE->Sync semaphore hop).
  - The last two tiles are split (loads AND compute: halves for t=14,
    quarters for t=15) so the final compute piece starts on the last
    512 KiB rather than the last 2 MiB.
"""

import numpy as np

import concourse.bacc as bacc
import concourse.mybir as mybir
from concourse.bass_utils import run_bass_kernel_spmd
from concourse.masks import make_identity
from concourse.tile import TileContext

N_CORES = 8
BATCH = 16384
IN_SIZE = 4096
P = 128
B_PER_CORE = BATCH // N_CORES  # 2048
N_TILES = B_PER_CORE // P  # 16

_NC_CACHE = None


def _build():
    nc = bacc.Bacc(
        "TRN2",
        target_bir_lowering=False,
        debug=False,
        num_devices=N_CORES,
    )
    x = nc.dram_tensor(
        "x", [B_PER_CORE, IN_SIZE], mybir.dt.float32, kind="ExternalInput"
    )
    w = nc.dram_tensor("w", [IN_SIZE], mybir.dt.float32, kind="ExternalInput")
    b = nc.dram_tensor("b", [1], mybir.dt.float32, kind="ExternalInput")
    y = nc.dram_tensor("y", [B_PER_CORE, 1], mybir.dt.float32, kind="ExternalOutput")

    xt = x.rearrange("(t p) k -> t p k", p=P)  # [16, 128, 4096]
    yv = y.rearrange("(t p) o -> t (p o)", p=P)  # [16, 128], 512B rows

    N_A = 15  # tiles 0..14 emitted mid-stream; tile 15 at the end
    N_B = N_TILES - N_A

    with TileContext(nc) as tc:
        with (
            tc.tile_pool(name="xpool", bufs=8) as xpool,
            tc.tile_pool(name="scratch", bufs=1) as spool,
            tc.tile_pool(name="consts", bufs=1) as cpool,
            tc.tile_pool(name="psum", bufs=1, space="PSUM") as ppool,
        ):
            x_tiles = {}

            def ring(t):
                return nc.sync if t % 2 == 0 else nc.scalar

            def load_x(t):
                x_PK = xpool.tile([P, IN_SIZE], mybir.dt.float32)
                ring(t).dma_start(out=x_PK[:], in_=xt[t])
                x_tiles[t] = x_PK

            # w leads the scalar ring (16 KiB, needed by the broadcast at
            # ~8 us), then ALL first-lap x loads (tiles 0..7, both rings,
            # free slots) are issued up front: HWDGE descgen executes on
            # the issuing sequencer IN PROGRAM ORDER, and in an earlier
            # rev the scalar-ring x descgens sat behind the w-broadcast
            # PSUM->SBUF copies (each stalled on its TensorE matmul), so
            # the scalar half of the stream started ~20 us late and
            # finished at ~115 us. With 16 MiB of descriptors queued on
            # both rings by ~13 us the 16 SDMA engines drain ~2 MiB each
            # continuously (~80 us, packet-interleaved across the two
            # queues); second-lap loads (t8+) are emitted as their ring
            # slots free, while the engines still have backlog.
            w_1K = cpool.tile([1, IN_SIZE], mybir.dt.float32)
            nc.scalar.dma_start(out=w_1K[:], in_=w[None, :])
            for t in range(8):
                load_x(t)
            b_11 = cpool.tile([1, 1], mybir.dt.float32)
            nc.scalar.dma_start(out=b_11[:], in_=b[None, :])
            ones_1P = cpool.tile([1, P], mybir.dt.float32)
            nc.vector.memset(ones_1P[:], 1.0)

            acc_PT = cpool.tile([P, N_TILES], mybir.dt.float32)
            NSPLIT = 4
            NQT = 4  # tiles that use the quarter-split
            STAGGER = 3
            KQ = IN_SIZE // NSPLIT
            accs_q = [
                cpool.tile([P, NQT], mybir.dt.float32, name=f"acc_{q}")
                for q in range(1, NSPLIT)
            ]
            acc_last = cpool.tile([P, 4], mybir.dt.float32)

            w_PK = cpool.tile([P, IN_SIZE], mybir.dt.float32)
            NCHUNK = 512
            for c in range(IN_SIZE // NCHUNK):
                cs = slice(c * NCHUNK, (c + 1) * NCHUNK)
                w_psum = ppool.tile([P, NCHUNK], mybir.dt.float32, bufs=2)
                nc.tensor.matmul(w_psum[:], ones_1P[:], w_1K[0:1, cs])
                nc.scalar.copy(w_PK[:, cs], w_psum[:])
            b_psum = ppool.tile([P, 1], mybir.dt.float32)
            nc.tensor.matmul(b_psum[:], ones_1P[:], b_11[:])
            b_P1 = cpool.tile([P, 1], mybir.dt.float32)
            nc.scalar.copy(b_P1[:], b_psum[:])
            ident = cpool.tile([P, P], mybir.dt.float32)
            make_identity(nc, ident[:])

            prod_PK = spool.tile([P, IN_SIZE], mybir.dt.float32)

            # The first 4 tiles are split into quarter-K ops with a
            # staggered emission (quarter q of tile t at step t + 3q):
            # quarter q only needs w[q*1024:(q+1)*1024], so DVE starts as
            # soon as the first w chunks are broadcast instead of waiting
            # for all of w. The Tile scheduler keeps same-engine program
            # order, so the stagger must be explicit.
            def emit_quarter(t, q):
                seg = slice(q * KQ, (q + 1) * KQ)
                acc = acc_PT[:, t : t + 1] if q == 0 else accs_q[q - 1][:, t : t + 1]
                nc.vector.affine_mul_reduce(
                    out=prod_PK[:, seg],
                    accum_out=acc,
                    in0=x_tiles[t][:, seg],
                    in1=w_PK[:, seg],
                    scale=1.0,
                    bias=0.0,
                )

            for i in range(NQT + STAGGER * (NSPLIT - 1)):
                if i < NQT:
                    emit_quarter(i, 0)
                for q in range(1, NSPLIT):
                    t = i - STAGGER * q
                    if 0 <= t < NQT:
                        emit_quarter(t, q)
            # Fold the quarter partials early (DVE program order!) so the
            # mid-stream chunk-A output only waits on tile 13's affine.
            for acc_q in accs_q:
                nc.vector.tensor_add(acc_PT[:, 0:NQT], acc_PT[:, 0:NQT], acc_q[:])

            # Mid tiles. Two are offloaded off the (binding) VectorE:
            # GpSimd does the elementwise multiply, ScalarE reduces it via
            # activation-accum. Both engines are otherwise idle mid-kernel
            # and finish long before their results are needed. The
            # offloaded tiles MUST be >= 8: with an 8-buffer x ring, slots
            # of tiles 8..15 are never reused, so GpSimd's ~11 us hold of
            # its x tile cannot block a later load.
            GPS_TILES = (8, 10)
            prod2_PK = spool.tile(
                [P, IN_SIZE], mybir.dt.float32, name="prod2_PK", tag="prod2"
            )
            for t in range(NQT, N_TILES - 2):
                if t >= 8:
                    load_x(t)
                if t in GPS_TILES:
                    nc.gpsimd.tensor_mul(prod2_PK[:], x_tiles[t][:], w_PK[:])
                    nc.scalar.activation(
                        prod2_PK[:],
                        prod2_PK[:],
                        mybir.ActivationFunctionType.Copy,
                        accum_out=acc_PT[:, t : t + 1],
                    )
                    continue
                nc.vector.affine_mul_reduce(
                    out=prod_PK[:],
                    accum_out=acc_PT[:, t : t + 1],
                    in0=x_tiles[t][:],
                    in1=w_PK[:],
                    scale=1.0,
                    bias=0.0,
                )

            # The last two tiles are split (loads AND compute) so the
            # final compute piece starts on the last 256 KiB rather than
            # the last 2 MiB. Segment s of tile t uses t's ring.
            def split_tile(t, segs, acc_off):
                x_PK = xpool.tile([P, IN_SIZE], mybir.dt.float32)
                x_tiles[t] = x_PK
                k0 = 0
                for s, seg_k in enumerate(segs):
                    seg = slice(k0, k0 + seg_k)
                    k0 += seg_k
                    ring(t).dma_start(out=x_PK[:, seg], in_=xt[t][:, seg])
                    nc.vector.affine_mul_reduce(
                        out=prod_PK[:, seg],
                        accum_out=acc_last[:, acc_off + s : acc_off + s + 1],
                        in0=x_PK[:, seg],
                        in1=w_PK[:, seg],
                        scale=1.0,
                        bias=0.0,
                    )

            t14, t15 = N_TILES - 2, N_TILES - 1
            split_tile(t14, (2048, 2048), 0)
            nc.vector.tensor_add(
                acc_PT[:, t14 : t14 + 1], acc_last[:, 0:1], acc_last[:, 1:2]
            )

            # Chunk A: tiles 0..14 go tanh -> transpose -> DMA as soon as
            # tile 14's combine lands, fully hidden under the tail of
            # the x stream. No DVE ops here — ScalarE/TensorE only.
            y_A = cpool.tile([P, N_A], mybir.dt.float32)
            nc.scalar.activation(
                y_A[:],
                acc_PT[:, 0:N_A],
                mybir.ActivationFunctionType.Tanh,
                bias=b_P1[:],
            )
            nc.scalar.activation(y_A[:], y_A[:], mybir.ActivationFunctionType.Tanh)
            yps_A = ppool.tile([N_A, P], mybir.dt.float32)
            nc.tensor.transpose(yps_A[:], y_A[:], ident[:])
            y_TA = cpool.tile([N_A, P], mybir.dt.float32)
            nc.scalar.copy(y_TA[:], yps_A[:])
            nc.scalar.dma_start(out=yv[0:N_A], in_=y_TA[:])

            # Tile 15: quarters then eighths, so the last affine covers
            # only 512 K-columns (256 KiB of x, ~0.65 us on DVE). Each
            # segment's partial goes to acc_last col 3 and is folded into
            # the col-2 running sum immediately (the fold adds are
            # emitted BETWEEN the affines so they execute in the DVE
            # idle gaps between segment arrivals); after the last affine
            # only one [128,1] add remains before the output chain.
            # name matches load_x's tiles so it shares their 8-slot ring
            # (untagged pool tiles are slotted by inferred variable name).
            x15 = xpool.tile([P, IN_SIZE], mybir.dt.float32, name="x_PK")
            k0 = 0
            for s, seg_k in enumerate((1024, 1024, 512, 512, 512, 512)):
                seg = slice(k0, k0 + seg_k)
                k0 += seg_k
                ring(t15).dma_start(out=x15[:, seg], in_=xt[t15][:, seg])
                col = 2 if s == 0 else 3
                nc.vector.affine_mul_reduce(
                    out=prod_PK[:, seg],
                    accum_out=acc_last[:, col : col + 1],
                    in0=x15[:, seg],
                    in1=w_PK[:, seg],
                    scale=1.0,
                    bias=0.0,
                )
                if 0 < s < 5:
                    nc.vector.tensor_add(
                        acc_last[:, 2:3], acc_last[:, 2:3], acc_last[:, 3:4]
                    )
            nc.vector.tensor_add(
                acc_PT[:, t15 : t15 + 1], acc_last[:, 2:3], acc_last[:, 3:4]
            )

            # Chunk B: only the 1-column tail. tanh(tanh(acc + b)) on
            # ScalarE (the DVE->ACT handoff needs no DVE drain), TensorE
            # transpose [128, 1] -> [1, 128], one 512 B DMA from the
            # scalar ring.
            y_B = cpool.tile([P, N_B], mybir.dt.float32)
            nc.scalar.activation(
                y_B[:],
                acc_PT[:, N_A:N_TILES],
                mybir.ActivationFunctionType.Tanh,
                bias=b_P1[:],
            )
            nc.scalar.activation(y_B[:], y_B[:], mybir.ActivationFunctionType.Tanh)
            yps_B = ppool.tile([N_B, P], mybir.dt.float32)
            nc.tensor.transpose(yps_B[:], y_B[:], ident[:])
            y_TB = cpool.tile([N_B, P], mybir.dt.float32)
            nc.scalar.copy(y_TB[:], yps_B[:])
            nc.scalar.dma_start(out=yv[N_A:N_TILES], in_=y_TB[:])
    nc.compile()
    return nc


def _get_nc():
    global _NC_CACHE
    if _NC_CACHE is None:
        _NC_CACHE = _build()
    return _NC_CACHE


def _run(x, w, b, **spmd_kwargs):
    """Shard, execute on 8 cores, gather. Returns (out, BassKernelResults)."""
    x = np.ascontiguousarray(np.asarray(x, dtype=np.float32))
    w = np.ascontiguousarray(np.asarray(w, dtype=np.float32))
    b = np.ascontiguousarray(np.asarray(b, dtype=np.float32))
    assert x.shape == (BATCH, IN_SIZE), x.shape

    nc = _get_nc()
    in_maps = [
        {"x": x[c * B_PER_CORE : (c + 1) * B_PER_CORE], "w": w, "b": b}
        for c in range(N_CORES)
    ]
    res = run_bass_kernel_spmd(nc, in_maps, list(range(N_CORES)), **spmd_kwargs)
    out = np.concatenate(
        [np.asarray(res.results[c]["y"]) for c in range(N_CORES)], axis=0
    )
    return out.astype(np.float32, copy=False), res


def kernel(x, w, b):
    try:
        out, _ = _run(x, w, b)
    except Exception:
        # Transient device-wedge (NRT_EXEC_UNIT_UNRECOVERABLE) has been
        # observed once on a first run and succeeded on retry.
        out, _ = _run(x, w, b)
    return out


# revision 18
# speedup vs baseline: 1.0585x; 1.0585x over previous
"""Trainium2 Bass kernel for CartNN minimal-NEAT forward pass.

Computes out = tanh(tanh(x @ w + b))[:, None] for x [16384, 4096] f32,
w [4096] f32, b [1] f32, data-parallel across 8 NeuronCores (2048 batch
rows per core). Memory-bound: each core streams its 32 MiB x shard once.

Per-core structure (measured on HW, iterated via NTFF profiles):
  - x streams as 16 [128, 4096] tiles, even tiles on the sync HWDGE
    ring, odd tiles via SWDGE (gpsimd) — two independent descriptor
    generators feeding the same 16 SDMA engines, testing whether SDMA
    engine 15's ~20% under-load slowdown (seen on both HWDGE rings)
    spares the SWDGE path. All x DMAs write full 128-partition tiles:
    partition-sliced destinations halve the per-descriptor rate.
  - ALL 9 first-lap tile loads are issued before any compute op is
    emitted on their sequencers: HWDGE/SWDGE descgen executes on the
    issuing sequencer in program order, and a sequencer stalled on a
    compute op's semaphore (e.g. a PSUM->SBUF copy waiting on its
    matmul, or an activation waiting on GpSimd) starves the queue —
    this cost earlier revs 20-35 us of stream stall. Second-lap loads
    are emitted next; their slot-free waits pace them at DVE
    consumption rate (~4.6 us/tile), faster than the stream.
  - w is loaded once (16 KiB, scalar ring) and broadcast to all 128
    partitions by TensorE outer products ones[128,1] @ w[1,512]
    (PSUM->SBUF copies on ScalarE): no sync-ring involvement
    (stride-0 DRAM broadcast DMAs measurably poison the x stream).
  - The dot product is one fused mul+reduce VectorE op per tile
    (affine_mul_reduce, ~4.55 us; TensorTensorReduce crashes the
    device). No GpSimd offload: a concurrent gpsimd tensor_mul
    measurably slows DVE affines 30-45% (SBUF port contention), and at
    the ~6.1 us/tile engine-15-paced arrival rate DVE (4.55) keeps up
    on its own. Tiles 13/14 split in K-halves and tile 15 in
    quarters+eighths with a running-sum fold, so DVE tracks the last
    arrivals piece-by-piece and finishes ~1 us after the last byte.
  - Output in two chunks: tiles 0..12 tanh(tanh(.+b)) -> TensorE
    transpose -> one DMA mid-stream (hidden); the end chain is tiles
    13-15 only: [128,3] tanh x2, transpose, 1.5 KiB DMA on the scalar
    ring (ScalarE just wrote the data, skipping a semaphore hop).
"""

import numpy as np

import concourse.bacc as bacc
import concourse.mybir as mybir
from concourse.bass_utils import run_bass_kernel_spmd
from concourse.masks import make_identity
from concourse.tile import TileContext

N_CORES = 8
BATCH = 16384
IN_SIZE = 4096
P = 128
B_PER_CORE = BATCH // N_CORES  # 2048
N_TILES = B_PER_CORE // P  # 16

_NC_CACHE = None


def _build():
    nc = bacc.Bacc(
        "TRN2",
        target_bir_lowering=False,
        debug=False,
        num_devices=N_CORES,
    )
    x = nc.dram_tensor(
        "x", [B_PER_CORE, IN_SIZE], mybir.dt.float32, kind="ExternalInput"
    )
    w = nc.dram_tensor("w", [IN_SIZE], mybir.dt.float32, kind="ExternalInput")
    b = nc.dram_tensor("b", [1], mybir.dt.float32, kind="ExternalInput")
    y = nc.dram_tensor("y", [B_PER_CORE, 1], mybir.dt.float32, kind="ExternalOutput")

    xt = x.rearrange("(t p) k -> t p k", p=P)  # [16, 128, 4096]
    yv = y.rearrange("(t p) o -> t (p o)", p=P)  # [16, 128], 512B rows

    N_A = 13  # tiles 0..12 emitted mid-stream; 13..15 at the end
    N_B = N_TILES - N_A
    SEGS = {13: (2048, 2048), 14: (2048, 2048), 15: (1024, 1024, 512, 512, 512, 512)}

    with TileContext(nc) as tc:
        with (
            tc.tile_pool(name="xpool", bufs=9) as xpool,
            tc.tile_pool(name="scratch", bufs=1) as spool,
            tc.tile_pool(name="consts", bufs=1) as cpool,
            tc.tile_pool(name="psum", bufs=1, space="PSUM") as ppool,
        ):
            x_tiles = {}

            def ring(t):
                return nc.sync if t % 2 == 0 else nc.gpsimd

            def load_x(t):
                x_PK = xpool.tile([P, IN_SIZE], mybir.dt.float32, name="x_PK")
                x_tiles[t] = x_PK
                if t in SEGS:
                    k0 = 0
                    for seg_k in SEGS[t]:
                        seg = slice(k0, k0 + seg_k)
                        k0 += seg_k
                        ring(t).dma_start(out=x_PK[:, seg], in_=xt[t][:, seg])
                else:
                    ring(t).dma_start(out=x_PK[:], in_=xt[t])

            # w leads the scalar ring, then ALL first-lap x loads (free
            # slots) so both DMA queues have ~18 MiB of descriptors
            # before any sequencer touches a stallable compute op.
            w_1K = cpool.tile([1, IN_SIZE], mybir.dt.float32)
            nc.scalar.dma_start(out=w_1K[:], in_=w[None, :])
            for t in range(9):
                load_x(t)
            b_11 = cpool.tile([1, 1], mybir.dt.float32)
            nc.scalar.dma_start(out=b_11[:], in_=b[None, :])

            # identity for the TensorE transposes; emitted on the gpsimd
            # sequencer before its (slot-waiting) second-lap descgens.
            ident = cpool.tile([P, P], mybir.dt.float32)
            make_identity(nc, ident[:])

            # Second-lap loads: each waits (on its issuing sequencer) for
            # its ring slot, freed at DVE consumption pace — well before
            # the SDMA queues drain the first lap.
            for t in range(9, N_TILES):
                load_x(t)

            ones_1P = cpool.tile([1, P], mybir.dt.float32)
            nc.vector.memset(ones_1P[:], 1.0)
            acc_PT = cpool.tile([P, N_TILES], mybir.dt.float32)
            acc_last = cpool.tile([P, 6], mybir.dt.float32)

            w_PK = cpool.tile([P, IN_SIZE], mybir.dt.float32)
            NCHUNK = 512
            for c in range(IN_SIZE // NCHUNK):
                cs = slice(c * NCHUNK, (c + 1) * NCHUNK)
                w_psum = ppool.tile([P, NCHUNK], mybir.dt.float32, bufs=2)
                nc.tensor.matmul(w_psum[:], ones_1P[:], w_1K[0:1, cs])
                nc.scalar.copy(w_PK[:, cs], w_psum[:])
            b_psum = ppool.tile([P, 1], mybir.dt.float32)
            nc.tensor.matmul(b_psum[:], ones_1P[:], b_11[:])
            b_P1 = cpool.tile([P, 1], mybir.dt.float32)
            nc.scalar.copy(b_P1[:], b_psum[:])

            prod_PK = spool.tile([P, IN_SIZE], mybir.dt.float32)

            def affine(t, seg, acc):
                nc.vector.affine_mul_reduce(
                    out=prod_PK[:, seg],
                    accum_out=acc,
                    in0=x_tiles[t][:, seg],
                    in1=w_PK[:, seg],
                    scale=1.0,
                    bias=0.0,
                )

            def emit_chunk_a():
                # Chunk A: tiles 0..N_A-1 go tanh -> transpose -> DMA as
                # soon as tile N_A-1's accumulate lands, fully hidden
                # under the x stream tail. No DVE ops — ScalarE/TensorE.
                y_A = cpool.tile([P, N_A], mybir.dt.float32, name="y_A")
                nc.scalar.activation(
                    y_A[:],
                    acc_PT[:, 0:N_A],
                    mybir.ActivationFunctionType.Tanh,
                    bias=b_P1[:],
                )
                nc.scalar.activation(
                    y_A[:], y_A[:], mybir.ActivationFunctionType.Tanh
                )
                yps_A = ppool.tile([N_A, P], mybir.dt.float32, name="yps_A")
                nc.tensor.transpose(yps_A[:], y_A[:], ident[:])
                y_TA = cpool.tile([N_A, P], mybir.dt.float32, name="y_TA")
                nc.scalar.copy(y_TA[:], yps_A[:])
                nc.scalar.dma_start(out=yv[0:N_A], in_=y_TA[:])

            # Full-K dot products for tiles 0..12.
            for t in range(N_A):
                affine(t, slice(0, IN_SIZE), acc_PT[:, t : t + 1])

            # Tiles 13/14: halves -> acc_last, one add each.
            for t, off in ((13, 0), (14, 2)):
                affine(t, slice(0, 2048), acc_last[:, off : off + 1])
                affine(t, slice(2048, 4096), acc_last[:, off + 1 : off + 2])
                nc.vector.tensor_add(
                    acc_PT[:, t : t + 1],
                    acc_last[:, off : off + 1],
                    acc_last[:, off + 1 : off + 2],
                )
                if t == 13:
                    emit_chunk_a()

            # Tile 15: running sum in col 4, incoming partial in col 5;
            # fold adds emitted BETWEEN the affines execute in the DVE
            # idle gaps between segment arrivals, so after the last
            # (512-col, ~0.65 us) affine only one [128,1] add remains.
            k0 = 0
            for s, seg_k in enumerate(SEGS[15]):
                seg = slice(k0, k0 + seg_k)
                k0 += seg_k
                col = 4 if s == 0 else 5
                affine(15, seg, acc_last[:, col : col + 1])
                if 0 < s < len(SEGS[15]) - 1:
                    nc.vector.tensor_add(
                        acc_last[:, 4:5], acc_last[:, 4:5], acc_last[:, 5:6]
                    )
            nc.vector.tensor_add(
                acc_PT[:, 15:16], acc_last[:, 4:5], acc_last[:, 5:6]
            )

            # Chunk B: the 3-column tail. tanh(tanh(acc + b)) on ScalarE
            # (the DVE->ACT handoff needs no DVE drain), TensorE
            # transpose, one 1.5 KiB DMA of 512-B rows (scalar ring).
            y_B = cpool.tile([P, N_B], mybir.dt.float32)
            nc.scalar.activation(
                y_B[:],
                acc_PT[:, N_A:N_TILES],
                mybir.ActivationFunctionType.Tanh,
                bias=b_P1[:],
            )
            nc.scalar.activation(y_B[:], y_B[:], mybir.ActivationFunctionType.Tanh)
            yps_B = ppool.tile([N_B, P], mybir.dt.float32)
            nc.tensor.transpose(yps_B[:], y_B[:], ident[:])
            y_TB = cpool.tile([N_B, P], mybir.dt.float32)
            nc.scalar.copy(y_TB[:], yps_B[:])
            nc.scalar.dma_start(out=yv[N_A:N_TILES], in_=y_TB[:])
    nc.compile()
    return nc


def _get_nc():
    global _NC_CACHE
    if _NC_CACHE is None:
        _NC_CACHE = _build()
    return _NC_CACHE


def _run(x, w, b, **spmd_kwargs):
    """Shard, execute on 8 cores, gather. Returns (out, BassKernelResults)."""
    x = np.ascontiguousarray(np.asarray(x, dtype=np.float32))
    w = np.ascontiguousarray(np.asarray(w, dtype=np.float32))
    b = np.ascontiguousarray(np.asarray(b, dtype=np.float32))
    assert x.shape == (BATCH, IN_SIZE), x.shape

    nc = _get_nc()
    in_maps = [
        {"x": x[c * B_PER_CORE : (c + 1) * B_PER_CORE], "w": w, "b": b}
        for c in range(N_CORES)
    ]
    res = run_bass_kernel_spmd(nc, in_maps, list(range(N_CORES)), **spmd_kwargs)
    out = np.concatenate(
        [np.asarray(res.results[c]["y"]) for c in range(N_CORES)], axis=0
    )
    return out.astype(np.float32, copy=False), res


def kernel(x, w, b):
    try:
        out, _ = _run(x, w, b)
    except Exception:
        # Transient device-wedge (NRT_EXEC_UNIT_UNRECOVERABLE) has been
        # observed once on a first run and succeeded on retry.
        out, _ = _run(x, w, b)
    return out


# revision 19
# speedup vs baseline: 1.3655x; 1.2901x over previous
"""Trainium2 Bass kernel for CartNN minimal-NEAT forward pass.

Computes out = tanh(tanh(x @ w + b))[:, None] for x [16384, 4096] f32,
w [4096] f32, b [1] f32, data-parallel across 8 NeuronCores (2048 batch
rows per core). Memory-bound: each core streams its 32 MiB x shard once.

Per-core structure (measured on HW, iterated via NTFF profiles):
  - x streams as 16 [128, 4096] tiles, ALL on the sync HWDGE ring.
    Measured alternatives all lose: SWDGE (gpsimd) tiles throttle BOTH
    paths to ~73% of line rate; a scalar-ring split leaves that queue
    descriptor-starved behind ScalarE compute ops; partition-sliced
    destinations (e.g. [0:120]) halve the per-descriptor rate. SDMA
    engine 15 runs ~20% slower whenever engine duty is ~100% (its
    sustained ceiling ~21.5 GB/s vs 26.4 line rate), which sets the
    stream wall at ~98 us busy for its 2 MiB share; the structure
    below keeps everything else off the critical path.
  - w loads first on the sync ring (16 KiB, lands ~5.7 us; on the
    scalar ring it lands at ~12 us and delays everything), then ALL 9
    first-lap x tile loads are issued before any compute op is emitted
    on any DMA-issuing sequencer: descgen executes on the issuing
    sequencer in program order, and a sequencer stalled on a compute
    semaphore starves its queue (cost earlier revs 20-35 us). Second-
    lap loads pace themselves on ring-slot frees at DVE consumption
    rate, well before the SDMA queues drain the first lap.
  - w is broadcast to 128 partitions by TensorE outer products
    ones[128,1] @ w[1,512] (PSUM->SBUF copies on ScalarE): no extra
    HBM traffic, no sync-ring involvement (stride-0 DRAM broadcast
    DMAs measurably poison the x stream).
  - The dot product is one fused mul+reduce VectorE op per tile
    (affine_mul_reduce, ~4.55 us; TensorTensorReduce crashes the
    device). No GpSimd offload: a concurrent gpsimd tensor_mul
    measurably slows DVE affines 30-45% (SBUF port contention), and at
    the ~6.1 us/tile engine-15-paced arrival rate DVE keeps up alone.
    Tiles 0/1 are split in K-halves with a 1-step stagger so DVE
    starts on w[0:2048] (~14 us) instead of full w (~19.5 us).
  - Tiles 13/14 split in K-halves and tile 15 in quarters+eighths with
    a running-sum fold emitted between the affines, so DVE tracks the
    last arrivals piece-by-piece: after the final 256 KiB lands only
    ~0.65 us of DVE + one [128,1] add remain.
  - Output in two chunks: tiles 0..12 tanh(tanh(.+b)) -> TensorE
    transpose -> one DMA mid-stream (fully hidden); the end chain is
    tiles 13-15 only: [128,3] tanh x2, transpose, 1.5 KiB DMA of 512-B
    rows on the scalar ring (ScalarE just wrote the data, skipping a
    semaphore hop).
"""

import numpy as np

import concourse.bacc as bacc
import concourse.mybir as mybir
from concourse.bass_utils import run_bass_kernel_spmd
from concourse.masks import make_identity
from concourse.tile import TileContext

N_CORES = 8
BATCH = 16384
IN_SIZE = 4096
P = 128
B_PER_CORE = BATCH // N_CORES  # 2048
N_TILES = B_PER_CORE // P  # 16

_NC_CACHE = None


def _build():
    nc = bacc.Bacc(
        "TRN2",
        target_bir_lowering=False,
        debug=False,
        num_devices=N_CORES,
    )
    x = nc.dram_tensor(
        "x", [B_PER_CORE, IN_SIZE], mybir.dt.float32, kind="ExternalInput"
    )
    w = nc.dram_tensor("w", [IN_SIZE], mybir.dt.float32, kind="ExternalInput")
    b = nc.dram_tensor("b", [1], mybir.dt.float32, kind="ExternalInput")
    y = nc.dram_tensor("y", [B_PER_CORE, 1], mybir.dt.float32, kind="ExternalOutput")

    xt = x.rearrange("(t p) k -> t p k", p=P)  # [16, 128, 4096]
    yv = y.rearrange("(t p) o -> t (p o)", p=P)  # [16, 128], 512B rows

    N_A = 13  # tiles 0..12 emitted mid-stream; 13..15 at the end
    N_B = N_TILES - N_A
    H = IN_SIZE // 2
    SEGS = {13: (H, H), 14: (H, H), 15: (1024, 1024, 512, 512, 512, 512)}

    with TileContext(nc) as tc:
        with (
            tc.tile_pool(name="xpool", bufs=9) as xpool,
            tc.tile_pool(name="scratch", bufs=1) as spool,
            tc.tile_pool(name="consts", bufs=1) as cpool,
            tc.tile_pool(name="psum", bufs=1, space="PSUM") as ppool,
        ):
            x_tiles = {}

            def load_x(t):
                x_PK = xpool.tile([P, IN_SIZE], mybir.dt.float32, name="x_PK")
                x_tiles[t] = x_PK
                if t in SEGS:
                    k0 = 0
                    for seg_k in SEGS[t]:
                        seg = slice(k0, k0 + seg_k)
                        k0 += seg_k
                        nc.sync.dma_start(out=x_PK[:, seg], in_=xt[t][:, seg])
                else:
                    nc.sync.dma_start(out=x_PK[:], in_=xt[t])

            # w first (single descriptor, ~0.7 us of sync-seq time, lands
            # ~5.7 us), then all first-lap x loads into the 9 free slots.
            w_1K = cpool.tile([1, IN_SIZE], mybir.dt.float32)
            nc.sync.dma_start(out=w_1K[:], in_=w[None, :])
            for t in range(9):
                load_x(t)
            b_11 = cpool.tile([1, 1], mybir.dt.float32)
            nc.scalar.dma_start(out=b_11[:], in_=b[None, :])

            # identity for the TensorE transposes (GpSimd ops, emitted
            # before nothing else needs that sequencer).
            ident = cpool.tile([P, P], mybir.dt.float32)
            make_identity(nc, ident[:])

            # Second-lap loads: each waits (on the sync sequencer) for
            # its ring slot, freed at DVE consumption pace — well before
            # the SDMA queues drain the first lap.
            for t in range(9, N_TILES):
                load_x(t)

            ones_1P = cpool.tile([1, P], mybir.dt.float32)
            nc.vector.memset(ones_1P[:], 1.0)
            acc_PT = cpool.tile([P, N_TILES], mybir.dt.float32)
            acc_half = cpool.tile([P, 2], mybir.dt.float32)
            acc_last = cpool.tile([P, 6], mybir.dt.float32)

            w_PK = cpool.tile([P, IN_SIZE], mybir.dt.float32)
            NCHUNK = 512
            for c in range(IN_SIZE // NCHUNK):
                cs = slice(c * NCHUNK, (c + 1) * NCHUNK)
                w_psum = ppool.tile([P, NCHUNK], mybir.dt.float32, bufs=2)
                nc.tensor.matmul(w_psum[:], ones_1P[:], w_1K[0:1, cs])
                nc.scalar.copy(w_PK[:, cs], w_psum[:])
            b_psum = ppool.tile([P, 1], mybir.dt.float32)
            nc.tensor.matmul(b_psum[:], ones_1P[:], b_11[:])
            b_P1 = cpool.tile([P, 1], mybir.dt.float32)
            nc.scalar.copy(b_P1[:], b_psum[:])

            prod_PK = spool.tile([P, IN_SIZE], mybir.dt.float32)

            def affine(t, seg, acc):
                nc.vector.affine_mul_reduce(
                    out=prod_PK[:, seg],
                    accum_out=acc,
                    in0=x_tiles[t][:, seg],
                    in1=w_PK[:, seg],
                    scale=1.0,
                    bias=0.0,
                )

            def emit_chunk_a():
                # Chunk A: tiles 0..N_A-1 go tanh -> transpose -> DMA as
                # soon as tile N_A-1's accumulate lands, fully hidden
                # under the x stream tail. No DVE ops — ScalarE/TensorE.
                y_A = cpool.tile([P, N_A], mybir.dt.float32, name="y_A")
                nc.scalar.activation(
                    y_A[:],
                    acc_PT[:, 0:N_A],
                    mybir.ActivationFunctionType.Tanh,
                    bias=b_P1[:],
                )
                nc.scalar.activation(
                    y_A[:], y_A[:], mybir.ActivationFunctionType.Tanh
                )
                yps_A = ppool.tile([N_A, P], mybir.dt.float32, name="yps_A")
                nc.tensor.transpose(yps_A[:], y_A[:], ident[:])
                y_TA = cpool.tile([N_A, P], mybir.dt.float32, name="y_TA")
                nc.scalar.copy(y_TA[:], yps_A[:])
                nc.scalar.dma_start(out=yv[0:N_A], in_=y_TA[:])

            # Tiles 0/1: K-halves with a 1-step stagger — half h needs
            # only w[h*2048:(h+1)*2048] (broadcast chunks 4h..4h+3), so
            # DVE starts ~5 us before the full w_PK is ready.
            affine(0, slice(0, H), acc_PT[:, 0:1])
            affine(1, slice(0, H), acc_PT[:, 1:2])
            affine(0, slice(H, IN_SIZE), acc_half[:, 0:1])
            affine(1, slice(H, IN_SIZE), acc_half[:, 1:2])
            nc.vector.tensor_add(acc_PT[:, 0:2], acc_PT[:, 0:2], acc_half[:])

            # Full-K dot products for tiles 2..12.
            for t in range(2, N_A):
                affine(t, slice(0, IN_SIZE), acc_PT[:, t : t + 1])

            # Tiles 13/14: halves -> acc_last, one add each.
            for t, off in ((13, 0), (14, 2)):
                affine(t, slice(0, H), acc_last[:, off : off + 1])
                affine(t, slice(H, IN_SIZE), acc_last[:, off + 1 : off + 2])
                nc.vector.tensor_add(
                    acc_PT[:, t : t + 1],
                    acc_last[:, off : off + 1],
                    acc_last[:, off + 1 : off + 2],
                )
                if t == 13:
                    emit_chunk_a()

            # Tile 15: running sum in col 4, incoming partial in col 5;
            # fold adds emitted BETWEEN the affines execute in the DVE
            # idle gaps between segment arrivals, so after the last
            # (512-col, ~0.65 us) affine only one [128,1] add remains.
            k0 = 0
            for s, seg_k in enumerate(SEGS[15]):
                seg = slice(k0, k0 + seg_k)
                k0 += seg_k
                col = 4 if s == 0 else 5
                affine(15, seg, acc_last[:, col : col + 1])
                if 0 < s < len(SEGS[15]) - 1:
                    nc.vector.tensor_add(
                        acc_last[:, 4:5], acc_last[:, 4:5], acc_last[:, 5:6]
                    )
            nc.vector.tensor_add(
                acc_PT[:, 15:16], acc_last[:, 4:5], acc_last[:, 5:6]
            )

            # Chunk B: the 3-column tail. tanh(tanh(acc + b)) on ScalarE
            # (the DVE->ACT handoff needs no DVE drain), TensorE
            # transpose, one 1.5 KiB DMA of 512-B rows (scalar ring).
            y_B = cpool.tile([P, N_B], mybir.dt.float32)
            nc.scalar.activation(
                y_B[:],
                acc_PT[:, N_A:N_TILES],
                mybir.ActivationFunctionType.Tanh,
                bias=b_P1[:],
            )
            nc.scalar.activation(y_B[:], y_B[:], mybir.ActivationFunctionType.Tanh)
            yps_B = ppool.tile([N_B, P], mybir.dt.float32)
            nc.tensor.transpose(yps_B[:], y_B[:], ident[:])
            y_TB = cpool.tile([N_B, P], mybir.dt.float32)
            nc.scalar.copy(y_TB[:], yps_B[:])
            nc.scalar.dma_start(out=yv[N_A:N_TILES], in_=y_TB[:])
    nc.compile()
    return nc


def _get_nc():
    global _NC_CACHE
    if _NC_CACHE is None:
        _NC_CACHE = _build()
    return _NC_CACHE


def _run(x, w, b, **spmd_kwargs):
    """Shard, execute on 8 cores, gather. Returns (out, BassKernelResults)."""
    x = np.ascontiguousarray(np.asarray(x, dtype=np.float32))
    w = np.ascontiguousarray(np.asarray(w, dtype=np.float32))
    b = np.ascontiguousarray(np.asarray(b, dtype=np.float32))
    assert x.shape == (BATCH, IN_SIZE), x.shape

    nc = _get_nc()
    in_maps = [
        {"x": x[c * B_PER_CORE : (c + 1) * B_PER_CORE], "w": w, "b": b}
        for c in range(N_CORES)
    ]
    res = run_bass_kernel_spmd(nc, in_maps, list(range(N_CORES)), **spmd_kwargs)
    out = np.concatenate(
        [np.asarray(res.results[c]["y"]) for c in range(N_CORES)], axis=0
    )
    return out.astype(np.float32, copy=False), res


def kernel(x, w, b):
    try:
        out, _ = _run(x, w, b)
    except Exception:
        # Transient device-wedge (NRT_EXEC_UNIT_UNRECOVERABLE) has been
        # observed once on a first run and succeeded on retry.
        out, _ = _run(x, w, b)
    return out
